# revision 11
# baseline (speedup 1.0000x reference)
"""Trainium2 Bass kernel for nn_Attention_12000138625343.

Full multi-head attention layer (B=2, S=2048, E=1024, H=16, hd=64, interleaved
RoPE on q/k, non-causal softmax) run tensor-parallel over 8 NeuronCores:

  - heads sharded 2-per-core (w1 columns / qkv projection sharded),
  - x replicated and host-cast to bf16, passed pre-transposed [E, B*S],
  - q/k stored bf16 after RoPE; scores computed transposed [k, q] in bf16
    with the two heads' K=64 matmuls packed into disjoint PE row-groups,
  - exp on ACT outputs bf16 probabilities; v is projected transposed,
    PE-transposed back (bf16 identity) into [k, hd+1] tiles whose ones
    column accumulates the softmax denominator during attn@v,
  - the divide runs off the TensorEngine (DVE reciprocal + DRAM-bounce
    broadcast + DVE multiply),
  - NO collectives: after each q-tile's divide, the core immediately runs
    its partial output projection (contraction over its own 128 channels,
    all 4096 rows) and DMAs the bf16 partial rows out; the host sums the
    8 cores' partials (the unshard step). This removes the 4 AllToAlls
    (~25-35us each) and their tail exposure entirely,
  - qkv chains for later r-tiles are dribbled into the attention unit
    loop on a deadline schedule (consumers of x tile n before tile n+3).

Measured on the fixture: see test.py; rel err ~1e-2 vs the 2e-2 gate.
"""

import math

import numpy as np

import concourse.bass as bass
import concourse.mybir as mybir
import concourse.tile as tile
from concourse import bacc
from concourse.bass_utils import run_bass_kernel_spmd
from concourse.masks import make_identity

B, S, E, H = 2, 2048, 1024, 16
HD = E // H  # 64
BASE = 10000.0
N_CORES = 8
HPC = H // N_CORES       # heads per core = 2
R = B * S                # 4096 flattened rows
RT = 512                 # rows per r-tile
NEC = E // 128           # 8 e-chunks of 128
QT = 512                 # q columns per q-tile
N_QT = S // QT           # 4 q-tiles per batch
KC = 128                 # k rows per k-chunk
N_KC = S // KC           # 16 k-chunks per batch

F32 = mybir.dt.float32
F32R = mybir.dt.float32r
BF16 = mybir.dt.bfloat16
EXPF = mybir.ActivationFunctionType.Exp

_COMPILED = {}


def _build_nc():
    nc = bacc.Bacc("TRN2", target_bir_lowering=False, debug=False,
                   num_devices=N_CORES)

    xT = nc.dram_tensor("xT", [E, R], BF16, kind="ExternalInput").ap()
    wqT = nc.dram_tensor("wqT", [E, 128], BF16, kind="ExternalInput").ap()
    wkT = nc.dram_tensor("wkT", [E, 128], BF16, kind="ExternalInput").ap()
    wvT = nc.dram_tensor("wvT", [E, 128], BF16, kind="ExternalInput").ap()
    w2my = nc.dram_tensor("w2my", [128, E], BF16, kind="ExternalInput").ap()
    cosT = nc.dram_tensor("cosT", [128, S], F32, kind="ExternalInput").ap()
    sinT = nc.dram_tensor("sinT", [128, S], F32, kind="ExternalInput").ap()
    p2T = nc.dram_tensor("p2T", [128, 128], F32, kind="ExternalInput").ap()
    out = nc.dram_tensor("out", [R, E], BF16, kind="ExternalOutput").ap()

    with tile.TileContext(nc) as tc:
        _emit(tc, nc, xT, wqT, wkT, wvT, w2my, cosT, sinT, p2T, out)
    nc.compile()
    return nc


def _emit(tc, nc, xT, wqT, wkT, wvT, w2my, cosT, sinT, p2T, out):
    import contextlib
    ctx = contextlib.ExitStack()
    consts = ctx.enter_context(tc.tile_pool(name="consts", bufs=1))
    xtp = ctx.enter_context(tc.tile_pool(name="xtp", bufs=4))
    qkp = ctx.enter_context(tc.tile_pool(name="qkp", bufs=1))
    rawp = ctx.enter_context(tc.tile_pool(name="rawp", bufs=2))
    tmpp = ctx.enter_context(tc.tile_pool(name="tmpp", bufs=2))
    vp = ctx.enter_context(tc.tile_pool(name="vp", bufs=1))
    pp = ctx.enter_context(tc.tile_pool(name="pp", bufs=7))
    smallp = ctx.enter_context(tc.tile_pool(name="smallp", bufs=2))
    otp = ctx.enter_context(tc.tile_pool(name="otp", bufs=3))
    dramp = ctx.enter_context(tc.tile_pool(name="dramp", bufs=1, space="DRAM"))
    # PSUM budget (8 banks): qkv/proj 2 + sps 2 x 2 + av 2 = 8
    ps_qkv = ctx.enter_context(tc.tile_pool(name="ps_qkv", bufs=2, space="PSUM"))
    ps_sps = ctx.enter_context(tc.tile_pool(name="ps_sps", bufs=2, space="PSUM"))
    ps_av = ctx.enter_context(tc.tile_pool(name="ps_av", bufs=2, space="PSUM"))

    # ---- tiny constants first: the identity (gpsimd iota) must precede the
    # xt posts on the gpsimd ring or the v-transposes deadlock against a
    # blocked xt DMA ----
    ones_f32 = consts.tile([128, 64], F32, tag="ones32", name="ones_f32")
    nc.vector.memset(ones_f32[:], 1.0)
    ones_bf = consts.tile([128, 1], BF16, tag="onesb", name="ones_bf")
    nc.vector.tensor_copy(ones_bf[:], ones_f32[:, 0:1])
    id_sb = consts.tile([128, 128], F32, tag="idm", name="id_sb")
    make_identity(nc, id_sb[:])
    id_bf = consts.tile([128, 128], BF16, tag="idb", name="id_bf")
    nc.vector.tensor_copy(id_bf[:], id_sb[:])

    # ---- weight/x loads, k-chain inputs first so scores can start early.
    # Each batch-0 x r-tile is split across the DMA queues so multiple rings
    # pull HBM concurrently; cos/sin are loaded per 512-column chunk just in
    # time for each r-tile's RoPE ----
    xTr = xT.rearrange("(c p) r -> p c r", p=128)
    wk_all = consts.tile([128, NEC, 128], BF16, tag="wk", name="wk_all")
    nc.gpsimd.dma_start(out=wk_all[:], in_=wkT.rearrange("(c p) f -> p c f", p=128))
    wq_all = consts.tile([128, NEC, 128], BF16, tag="wq", name="wq_all")
    nc.sync.dma_start(out=wq_all[:],
                  in_=wqT.rearrange("(c p) f -> p c f", p=128))
    xts = {}
    cos_sb = consts.tile([128, S], F32, tag="cos", name="cos_sb")
    sin_sb = consts.tile([128, S], F32, tag="sin", name="sin_sb")
    p2_sb = consts.tile([128, 128], F32R, tag="p2", name="p2_sb")
    wv_all = consts.tile([128, NEC, 128], BF16, tag="wv", name="wv_all")
    w2_sb = consts.tile([128, E], BF16, tag="w2", name="w2_sb")

    def post_xt_split(rt, four=False):
        t = xtp.tile([128, NEC, RT], BF16, tag="xt", name=f"xt_{rt}")
        c0, c1 = rt * RT, (rt + 1) * RT
        if four:
            # first tiles gate the whole pipeline: pull them over several
            # DGE rings at once (each ring drains its descriptors serially)
            nc.gpsimd.dma_start(out=t[:, 0:3, :], in_=xTr[:, 0:3, c0:c1])
            nc.sync.dma_start(out=t[:, 3:5, :], in_=xTr[:, 3:5, c0:c1])
            nc.scalar.dma_start(out=t[:, 5:NEC, :], in_=xTr[:, 5:NEC, c0:c1])
        else:
            nc.gpsimd.dma_start(out=t[:, 0:4, :], in_=xTr[:, 0:4, c0:c1])
            nc.sync.dma_start(out=t[:, 4:NEC, :], in_=xTr[:, 4:NEC, c0:c1])
        xts[rt] = t
        return t

    def cossin(i):
        nc.gpsimd.dma_start(out=cos_sb[:, i * RT:(i + 1) * RT],
                            in_=cosT[:, i * RT:(i + 1) * RT])
        nc.gpsimd.dma_start(out=sin_sb[:, i * RT:(i + 1) * RT],
                            in_=sinT[:, i * RT:(i + 1) * RT])

    # sync ring carries only x halves (plus wq/p2): both DGE rings process
    # their descriptors serially, so RoPE tables must not delay x tiles
    post_xt_split(0, four=True)
    nc.sync.dma_start(out=p2_sb[:], in_=p2T[:, :].bitcast(F32R))
    nc.gpsimd.dma_start(out=wv_all[:], in_=wvT.rearrange("(c p) f -> p c f", p=128))
    cossin(0)
    post_xt_split(1, four=True)
    cossin(1)
    post_xt_split(2)
    cossin(2)
    post_xt_split(3)
    cossin(3)
    # w2 slice (256KB) on the scalar ring; needed from the first divide on
    nc.scalar.dma_start(out=w2_sb[:], in_=w2my[:, :].rearrange("p f -> p f"))

    qT_sb, kT_sb, v_sb = {}, {}, {}

    # v tiles created up front so their ones columns (softmax-denominator
    # accumulators, cols 64 and 129) are written once during the startup DMA
    # window instead of inside the attention loop
    for vb in range(B):
        for vkc in range(N_KC):
            vt = vp.tile([128, 130], BF16, tag=f"v{vb}{vkc}",
                         name=f"v{vb}{vkc}")
            nc.vector.tensor_copy(vt[:, 64:65], ones_bf[:, 0:1])
            nc.vector.tensor_copy(vt[:, 129:130], ones_bf[:, 0:1])
            v_sb[(vb, vkc)] = vt

    def emit_xt_load(rt):
        # batch-1 tiles load on gpsimd only: their posts block on xtp pool
        # reuse, and the sync queue must stay clear for the divide DMAs
        if rt in xts:
            return xts[rt]
        t = xtp.tile([128, NEC, RT], BF16, tag="xt", name=f"xt_{rt}")
        nc.gpsimd.dma_start(out=t[:], in_=xTr[:, :, rt * RT:(rt + 1) * RT])
        xts[rt] = t
        return t

    def qk_chain(kind, rt, dribbled):
        """Two closures emitting the q- or k-projection (+RoPE) for r-tile
        rt. Dribbled chains evict on DVE to keep ACT free for exp."""
        b, st = rt // N_QT, (rt % N_QT) * RT
        w_all = wq_all if kind == "q" else wk_all
        if b not in qT_sb:
            qT_sb[b] = qkp.tile([128, S], BF16, tag=f"qT{b}", name=f"qT{b}")
            kT_sb[b] = qkp.tile([128, S], BF16, tag=f"kT{b}", name=f"kT{b}")
        dst = qT_sb[b] if kind == "q" else kT_sb[b]
        state = {}

        def emit_a():
            xt = xts[rt]
            acc = ps_qkv.tile([128, RT], F32, tag="qkv", name=f"{kind}acc{rt}")
            for ec in range(4):
                nc.tensor.matmul(acc[:], w_all[:, ec, :], xt[:, ec, :],
                                 start=(ec == 0), stop=False)
            state["acc"] = acc

        def emit_b():
            xt = xts[rt]
            acc = state.pop("acc")
            for ec in range(4, NEC):
                nc.tensor.matmul(acc[:], w_all[:, ec, :], xt[:, ec, :],
                                 start=False, stop=(ec == NEC - 1))
            raw = rawp.tile([128, RT], F32R, tag="raw", name=f"{kind}raw{rt}")
            if dribbled:
                nc.vector.tensor_copy(raw[:], acc[:])
            else:
                nc.scalar.copy(raw[:], acc[:])
            rot = ps_qkv.tile([128, RT], F32, tag="qkv", name=f"{kind}rot{rt}")
            nc.tensor.matmul(rot[:], p2_sb[:], raw[:], start=True, stop=True)
            t1 = tmpp.tile([128, RT], F32, tag="ropet", name=f"{kind}t1_{rt}")
            nc.vector.tensor_mul(t1[:], raw[:].bitcast(F32),
                                 cos_sb[:, st:st + RT])
            t2 = tmpp.tile([128, RT], F32, tag="ropet", name=f"{kind}t2_{rt}")
            nc.vector.tensor_mul(t2[:], rot[:], sin_sb[:, st:st + RT])
            nc.vector.tensor_add(dst[:, st:st + RT], t1[:], t2[:])
        return [emit_a, emit_b]

    def v_chains(rt, dribbled):
        """Four closures for the v projection of r-tile rt: two matmul halves
        in transposed orientation, two transpose-back pairs."""
        b = rt // N_QT
        vstate = {}

        def head(half):
            def emit():
                xt = xts[rt]
                if half == 0:
                    vacc = ps_qkv.tile([128, RT], F32, tag="qkv",
                                       name=f"vTacc{rt}")
                    vstate["ps"] = vacc
                vacc = vstate["ps"]
                for ec in range(4 * half, 4 * half + 4):
                    nc.tensor.matmul(vacc[:], wv_all[:, ec, :], xt[:, ec, :],
                                     start=(ec == 0), stop=(ec == NEC - 1))
                if half == 1:
                    vts = rawp.tile([128, RT], BF16, tag="rawb", name=f"vts{rt}")
                    if dribbled:
                        nc.vector.tensor_copy(vts[:], vstate.pop("ps")[:])
                    else:
                        nc.scalar.copy(vts[:], vstate.pop("ps")[:])
                    vstate["sb"] = vts
            return emit

        def tail(pair):
            def emit():
                vts = vstate["sb"]
                for sub in (2 * pair, 2 * pair + 1):
                    vtr = ps_qkv.tile([128, 128], BF16, tag="qkv",
                                      name=f"vtr{rt}_{sub}")
                    nc.tensor.transpose(
                        vtr[:], vts[:, sub * 128:(sub + 1) * 128], id_bf[:])
                    kc = (rt % N_QT) * 4 + sub
                    vt = v_sb[(b, kc)]
                    nc.vector.tensor_copy(vt[:, 0:64], vtr[:, 0:64])
                    nc.vector.tensor_copy(vt[:, 65:129], vtr[:, 64:128])
            return emit

        return [head(0), head(1), tail(0), tail(1)]

    def emit_divide(b, qt, avs):
        """Divide by the softmax denominator (row 64 of av), then run this
        q-tile's partial output projection (contraction over my 128 chans)
        and DMA the bf16 partial rows out. PE-free divide: DVE reciprocal +
        Pool-engine partition broadcast; the multiply reads the av PSUM
        directly. Chunked per 128-row block so the projection starts as
        early as possible."""
        bcss = []
        for h in range(HPC):
            den = smallp.tile([1, QT], F32, tag="den", name=f"den{b}{h}{qt}")
            nc.vector.tensor_copy(den[:], avs[h][64:65, :])
            rcp = smallp.tile([1, QT], F32, tag="rcp", name=f"rcp{b}{h}{qt}")
            # reciprocal_approx_fast is a custom DVE op: in/out must sit at
            # base partition 0, hence the separate denominator eviction
            nc.vector.reciprocal_approx_fast(rcp[:], den[:])
            bcs = smallp.tile([64, QT], F32, tag="bcs", name=f"bcs{b}{h}{qt}")
            nc.gpsimd.partition_broadcast(bcs[:], rcp[:])
            bcss.append(bcs)
        odiv = smallp.tile([128, QT], BF16, tag="odiv", name=f"odiv{b}{qt}")

        # partial projection: out rows [b*2048 + qt*512 + rb*128, :]
        def proj(rb):
            def emit():
                c0, c1 = rb * 128, (rb + 1) * 128
                for h in range(HPC):
                    nc.vector.tensor_mul(odiv[h * 64:(h + 1) * 64, c0:c1],
                                         avs[h][0:64, c0:c1],
                                         bcss[h][:, c0:c1])
                ot = otp.tile([128, E], BF16, tag="ot", name=f"ot{b}{qt}_{rb}")
                for fh in range(2):
                    ops = ps_qkv.tile([128, RT], F32, tag="qkv",
                                      name=f"ops{b}{qt}_{rb}_{fh}")
                    nc.tensor.matmul(
                        ops[:],
                        odiv[:, c0:c1],
                        w2_sb[:, fh * 512:(fh + 1) * 512],
                        start=True, stop=True)
                    nc.vector.tensor_copy(ot[:, fh * 512:(fh + 1) * 512],
                                          ops[:])
                r0 = b * S + qt * QT + rb * 128
                ring = nc.sync if rb % 2 == 0 else nc.scalar
                ring.dma_start(out=out[r0:r0 + 128, :], in_=ot[:])
            return emit
        return [proj(rb) for rb in range(4)]

    def emit_attention_batch(b, sched):
        """All 4 q-tiles of a batch as one rolling pipeline over 64+LAG
        (qt, kc) units: scores+exp lead, attn@v trails by LAG units, the
        divide chain fires as each q-tile's accumulation completes. sched
        maps unit -> list of dribble closures popped at that unit's top."""
        scale = 1.0 / math.sqrt(HD)
        NU = N_QT * N_KC
        LAG = 5
        pts = {}
        avs = {}
        for u in range(NU + LAG):
            for chain in sched.pop(u, ()):
                chain()
            if u < NU:
                qt, kc = divmod(u, N_KC)
                if kc == 0:
                    avs[qt] = [ps_av.tile([65, QT], F32, tag="av",
                                          name=f"av{b}{h}{qt}")
                               for h in range(HPC)]
                sps = ps_sps.tile([128, 2 * QT], F32, tag="sps",
                                  name=f"s{b}{qt}_{kc}")
                for h in range(HPC):
                    hof = h * 64
                    nc.tensor.matmul(
                        sps[:, h * QT:(h + 1) * QT],
                        kT_sb[b][hof:hof + 64, kc * KC:(kc + 1) * KC],
                        qT_sb[b][hof:hof + 64, qt * QT:(qt + 1) * QT],
                        start=True, stop=True)
                pt = pp.tile([128, 2 * QT], BF16, tag="p", name=f"p{b}{qt}_{kc}")
                nc.scalar.activation(pt[:], sps[:], EXPF, scale=scale)
                pts[u] = pt
            if u >= LAG:
                j = u - LAG
                qt2, kc2 = divmod(j, N_KC)
                for h in range(HPC):
                    nc.tensor.matmul(avs[qt2][h][:],
                                     v_sb[(b, kc2)][:, h * 65:(h + 1) * 65],
                                     pts[j][:, h * QT:(h + 1) * QT],
                                     start=(kc2 == 0), stop=(kc2 == N_KC - 1))
                del pts[j]
                if kc2 == N_KC - 1:
                    for chain in emit_divide(b, qt2, avs.pop(qt2)):
                        chain()
        # leftovers (if the schedule ran past the unit count)
        for u in sorted(sched):
            for chain in sched.pop(u):
                chain()

    # ---------------- emission ----------------
    # batch-0 r-tile 0: k then q inline (scores(qt0, kc0..3) ready ASAP)
    for chain in qk_chain("k", 0, dribbled=False):
        chain()
    for chain in qk_chain("q", 0, dribbled=False):
        chain()

    # batch-0 dribble: deadline-scheduled remaining batch-0 chains (all
    # consumers of xt_n emitted before any consumer of xt_{n+3}, since the
    # xtp pool has 3 buffers), then batch-1 qkv from u=24 (1 pop/unit)
    sched = {}
    def put(u, *chains):
        sched.setdefault(u, []).extend(chains)
    v0 = v_chains(0, dribbled=True)
    k1 = qk_chain("k", 1, dribbled=True)
    v1 = v_chains(1, dribbled=True)
    q1 = qk_chain("q", 1, dribbled=True)
    k2 = qk_chain("k", 2, dribbled=True)
    k3 = qk_chain("k", 3, dribbled=True)
    v2 = v_chains(2, dribbled=True)
    q2 = qk_chain("q", 2, dribbled=True)
    v3 = v_chains(3, dribbled=True)
    q3 = qk_chain("q", 3, dribbled=True)
    put(1, v0[0], v0[1])
    put(2, k1[0])
    put(3, k1[1])
    put(4, v0[2])
    put(5, v0[3])
    put(6, k2[0], v1[0])
    put(7, k2[1], v1[1])
    put(8, q1[0])
    put(9, v1[2], q1[1])
    put(10, v1[3])
    put(11, k3[0])
    put(12, k3[1], v2[0])
    put(13, v2[1], v2[2])
    put(14, v2[3])
    put(15, q2[0])
    put(16, q2[1], v3[0])
    put(17, v3[1], v3[2])
    put(18, v3[3])
    put(19, q3[0])
    put(20, q3[1])
    # batch-1 qkv chains dribble into batch-0's window from u=24. xt posts
    # are scheduled closures so the gpsimd queue (which also carries the
    # divide broadcasts) never parks on an xtp-pool semaphore ahead of a
    # divide.
    put(12, lambda: emit_xt_load(4))
    put(18, lambda: emit_xt_load(5))
    put(24, lambda: emit_xt_load(6))
    put(32, lambda: emit_xt_load(7))
    b1_chains = []
    for rt in range(N_QT, 2 * N_QT):
        b1_chains.extend(qk_chain("k", rt, dribbled=True))
        if rt < 2 * N_QT - 2:
            b1_chains.extend(qk_chain("q", rt, dribbled=True))
            b1_chains.extend(v_chains(rt, dribbled=True))
    for i, chain in enumerate(b1_chains):
        put(24 + i, chain)
    emit_attention_batch(0, sched)

    # rt6/rt7's q/v ride in batch-1's own window (it has exp slack;
    # batch-0's window is PE-bound)
    sched = {}
    q6 = qk_chain("q", 6, dribbled=True)
    v6 = v_chains(6, dribbled=True)
    q7 = qk_chain("q", 7, dribbled=True)
    v7 = v_chains(7, dribbled=True)
    put(1, v6[0])
    put(2, v6[1])
    put(3, v6[2])      # b1 kc8,9 needed at u13
    put(4, v6[3])      # kc10,11 at u15
    put(5, v7[0])
    put(6, v7[1])
    put(7, v7[2])      # kc12,13 at u17
    put(8, v7[3])      # kc14,15 at u19
    put(9, q6[0])
    put(10, q6[1])     # scores qt2 at u32
    put(11, q7[0])
    put(12, q7[1])     # scores qt3 at u48
    emit_attention_batch(1, sched)
    ctx.close()


def _host_prep(x, w1, w2):
    import ml_dtypes
    bf16 = ml_dtypes.bfloat16
    x = np.ascontiguousarray(np.asarray(x, dtype=np.float32))
    w1 = np.ascontiguousarray(np.asarray(w1, dtype=np.float32))
    w2 = np.ascontiguousarray(np.asarray(w2, dtype=np.float32))

    xT = np.ascontiguousarray(x.reshape(R, E).T.astype(bf16))  # [E, R] bf16

    theta = 1.0 / (BASE ** (np.arange(0, HD, 2, dtype=np.float32) / HD))
    enc = np.arange(S, dtype=np.float32)[:, None] * theta[None, :]
    enc = np.repeat(enc, 2, axis=-1)                      # [s, 64]
    cos1 = np.cos(enc).T.astype(np.float32)               # [64, S]
    sin1 = np.sin(enc).T.astype(np.float32)
    cosT = np.ascontiguousarray(np.concatenate([cos1, cos1], axis=0))
    sinT = np.ascontiguousarray(np.concatenate([sin1, sin1], axis=0))

    m64 = np.zeros((HD, HD), dtype=np.float32)
    for i in range(HD // 2):
        m64[2 * i, 2 * i + 1] = -1.0
        m64[2 * i + 1, 2 * i] = 1.0
    m128 = np.zeros((128, 128), dtype=np.float32)
    m128[:64, :64] = m64
    m128[64:, 64:] = m64
    p2T = np.ascontiguousarray(m128.T)

    in_maps = []
    for c in range(N_CORES):
        e0 = 128 * c           # heads 2c, 2c+1 occupy chans [128c, 128c+128)
        in_maps.append({
            "xT": xT,
            "wqT": np.ascontiguousarray(w1[e0:e0 + 128, :].T.astype(bf16)),
            "wkT": np.ascontiguousarray(w1[E + e0:E + e0 + 128, :].T.astype(bf16)),
            "wvT": np.ascontiguousarray(
                w1[2 * E + e0:2 * E + e0 + 128, :].T.astype(bf16)),
            "w2my": np.ascontiguousarray(w2[:, e0:e0 + 128].T.astype(bf16)),
            "cosT": cosT,
            "sinT": sinT,
            "p2T": p2T,
        })
    return in_maps


def kernel(x, w1, w2, _trace=False):
    if "nc" not in _COMPILED:
        _COMPILED["nc"] = _build_nc()
    nc = _COMPILED["nc"]
    in_maps = _host_prep(x, w1, w2)
    res = run_bass_kernel_spmd(nc, in_maps, core_ids=list(range(N_CORES)),
                               trace=_trace)
    _COMPILED["last_result"] = res
    # each core returns its partial projection over its own 128 channels
    # for all 4096 rows; the full output is the sum of the 8 partials
    acc = np.zeros((R, E), dtype=np.float32)
    for c in range(N_CORES):
        acc += res.results[c]["out"].astype(np.float32)
    return acc.reshape(B, S, E)
